# revision 1
# baseline (speedup 1.0000x reference)
"""EntropyInvarianceAttention Trainium2 Bass kernel.

Full inputs: q,k,v (4, 512, 2048) f32, k_length (4,) int32.
out = softmax_k(s_b * q^T k) @ v per (b, h) pair, s_b = log(k_length_b)/(8*log 20).

Sharding: 32 (b,h) pairs -> 8 cores x 4 pairs (core c: batch c//2, heads 4*(c%2)..+4).

Per-core kernel (flash-style, scores never touch HBM):
  - S^T = K^T Q per 128-k-tile via fp32r matmuls (k on partitions, q on free dim)
  - P^T = exp(s*S^T - 20) on ACT (scale/bias as per-partition APs) -> bf16
    (fixed shift: bf16 range covers the full score spread; shift cancels in
     normalization, so no per-row max pass is needed)
  - O_unnorm/denominator in one matmul: lhsT = [V^T | 1] (128k, 65) bf16,
    accumulate over 16 k-tiles into PSUM (65, 512q); row 64 = denominator
  - normalize: reciprocal of denom row, broadcast via ones(1,64)^T matmul,
    multiply on DVE, DMA out
  - V^T built by bf16 convert -> DRAM bounce -> XBAR transpose DMA loads
PSUM: 2x(128,1024) score slots + 4x(65,512) AV accumulators = 8 banks.
"""
import sys
import numpy as np
from contextlib import ExitStack

sys.path.insert(0, "/opt/trn_rl_repo")

import concourse.bass as bass
import concourse.tile as tile
from concourse import bacc, mybir
from concourse.bass_utils import run_bass_kernel_spmd

F32 = mybir.dt.float32
F16 = mybir.dt.float16
BF16 = mybir.dt.bfloat16
AF = mybir.ActivationFunctionType

B, H, D, L = 4, 8, 64, 2048
N_CORES = 8
PAIRS = 4                 # (b,h) pairs per core
ROWS = PAIRS * D          # 256 rows of q/k/v per core
KT = L // 128             # 16 k-tiles
QC = L // 512             # 4 q-chunks
SCALE = 1.0 / (D ** 0.5 * float(np.log(20.0)))
C_SHIFT = 20.0            # exp(s*score - C); cancels in softmax normalization


def _emit_prep(nc, pools, p, aps):
    """Loads + V^T construction for pair p."""
    kq, vv, vtp = pools["kq"], pools["vv"], pools["vt"]
    q_d, k_d, v_d = aps["q"], aps["k"], aps["v"]
    r0 = p * D

    k32 = kq.tile([D, L], F32, tag="k32")
    nc.sync.dma_start(out=k32[:], in_=k_d[r0:r0 + D, :])
    kr = kq.tile([D, L], F16, tag="kr")
    nc.vector.tensor_copy(out=kr[:], in_=k32[:])
    q32 = kq.tile([D, L], F32, tag="q32")
    nc.sync.dma_start(out=q32[:], in_=q_d[r0:r0 + D, :])
    qr = kq.tile([D, L], F16, tag="qr")
    nc.vector.tensor_copy(out=qr[:], in_=q32[:])
    v32 = vv.tile([D, L], F32, tag="v32")
    nc.sync.dma_start(out=v32[:], in_=v_d[r0:r0 + D, :])
    v16 = vv.tile([D, L], BF16, tag="v16")
    # scale by 2^-32 during the convert: the AV numerator then carries the
    # same 2^-32 as the denominator (ones column), so rec = 1/(d*2^-32)
    # normalizes exactly with no extra rescale op
    nc.vector.tensor_scalar_mul(out=v16[:], in0=v32[:], scalar1=2.0 ** -32)

    vt = vtp.tile([128, KT, 128], BF16, tag="vt")
    nc.vector.memset(vt[:, :, D:D + 1], 2.0 ** -32)  # scaled ones -> denominator*2^-32
    # single-instruction SBUF->SBUF XBAR transpose into the 3D tile
    nc.sync.dma_start(out=vt[:, :, 0:D], in_=v16[:], transpose=True)
    return kr, qr, vt


def _emit_norm(nc, pools, p, acc, aps, consts):
    """Normalize pair p's accumulator tile (65, 2048) and store the output.

    No PE involvement: reciprocal on DVE, partition-broadcast via a DRAM
    round trip (stride-0 partition APs are DRAM-only). The acc tile frees
    as soon as the four copies are done.
    """
    npo, outp = pools["np"], pools["out"]
    out_d, rec_d = aps["out"], aps["rec"]
    r0 = p * D

    out_t = outp.tile([D, L], F32, tag="out")
    av_sb = npo.tile([D + 1, L], F32, tag="avsb")
    nc.vector.tensor_copy(out=av_sb[:], in_=acc[:])
    den0 = npo.tile([1, L], F32, tag="den0")
    nc.sync.dma_start(out=den0[:], in_=av_sb[D:D + 1, :])
    rec = npo.tile([1, L], F32, tag="rec")
    # denom row holds d*2^-32 (safe range for the fast recip seed)
    nc.vector.reciprocal_approx_fast(out=rec[:], in_=den0[:])
    ri = QC * p
    nc.sync.dma_start(out=rec_d[ri:ri + QC, :], in_=rec[:])
    for c in range(QC):
        sl = slice(512 * c, 512 * (c + 1))
        bc = npo.tile([D, 512], F32, tag="bc")
        rb = bass.AP(tensor=rec_d.tensor,
                     offset=rec_d[ri + c:ri + c + 1, :].offset,
                     ap=[[0, D]] + [list(a) for a in
                                    rec_d[ri + c:ri + c + 1, :].ap[1:]])
        nc.sync.dma_start(out=bc[:], in_=rb)
        nc.vector.tensor_mul(out=out_t[:, sl], in0=av_sb[0:D, sl], in1=bc[:])
    nc.sync.dma_start(out=out_d[r0:r0 + D, :], in_=out_t[:])


def build():
    nc = bacc.Bacc("TRN2", target_bir_lowering=False, debug=False)
    aps = {
        "q": nc.dram_tensor("q", [ROWS, L], F32, kind="ExternalInput").ap(),
        "k": nc.dram_tensor("k", [ROWS, L], F32, kind="ExternalInput").ap(),
        "v": nc.dram_tensor("v", [ROWS, L], F32, kind="ExternalInput").ap(),
        "kl": nc.dram_tensor("kl", [1, 1], F32, kind="ExternalInput").ap(),
        "out": nc.dram_tensor("out", [ROWS, L], F32, kind="ExternalOutput").ap(),
        "rec": nc.dram_tensor("recs", [PAIRS * QC, 512], F32).ap(),  # scratch
    }

    with tile.TileContext(nc) as tc, ExitStack() as ctx:
        pools = {
            "kq": ctx.enter_context(tc.tile_pool(name="kq", bufs=2)),
            "vv": ctx.enter_context(tc.tile_pool(name="vv", bufs=2)),
            "vt": ctx.enter_context(tc.tile_pool(name="vt", bufs=2)),
            "pt": ctx.enter_context(tc.tile_pool(name="pt", bufs=6)),
            "out": ctx.enter_context(tc.tile_pool(name="out", bufs=2)),
            "np": ctx.enter_context(tc.tile_pool(name="np", bufs=3)),
            "cst": ctx.enter_context(tc.tile_pool(name="cst", bufs=1)),
            "sc": ctx.enter_context(tc.tile_pool(name="sc", bufs=2, space="PSUM")),
            "av": ctx.enter_context(tc.tile_pool(name="av", bufs=1, space="PSUM")),
        }
        cst = pools["cst"]

        # s = SCALE * ln(k_length), broadcast to all 128 partitions
        kl_b = cst.tile([128, 1], F32)
        kl_bcast = bass.AP(tensor=aps["kl"].tensor, offset=aps["kl"].offset,
                           ap=[[0, 128], [1, 1]])
        nc.sync.dma_start(out=kl_b[:], in_=kl_bcast)
        s128 = cst.tile([128, 1], F32)
        nc.scalar.activation(out=s128[:], in_=kl_b[:], func=AF.Ln)
        nc.vector.tensor_scalar_mul(out=s128[:], in0=s128[:], scalar1=SCALE)
        negc = cst.tile([128, 1], F32)
        nc.vector.memset(negc[:], -C_SHIFT)
        consts = {}

        prep = {0: _emit_prep(nc, pools, 0, aps)}
        prev_acc = None
        for p in range(PAIRS):
            kr, qr, vt = prep.pop(p)
            ptk_list = []
            acc = None
            for t in range(KT):
                ptk = pools["pt"].tile([128, L], BF16, tag="ptk")
                ptk_list.append(ptk)
                for h in range(2):
                    sc = pools["sc"].tile([128, 1024], F32, tag="sc")
                    for j in range(2):
                        q0 = 1024 * h + 512 * j
                        nc.tensor.matmul(
                            out=sc[:, 512 * j:512 * (j + 1)],
                            lhsT=kr[:, 128 * t:128 * (t + 1)],
                            rhs=qr[:, q0:q0 + 512],
                            start=True, stop=True)
                    nc.scalar.activation(
                        out=ptk[:, 1024 * h:1024 * (h + 1)], in_=sc[:],
                        func=AF.Exp, bias=negc[:], scale=s128[:])
                if t == 0 and p + 1 < PAIRS:
                    # prefetch next pair while this pair computes
                    prep[p + 1] = _emit_prep(nc, pools, p + 1, aps)
                if t == 1:
                    if prev_acc is not None:
                        _emit_norm(nc, pools, p - 1, prev_acc, aps, consts)
                    acc = pools["av"].tile([D + 1, L], F32, tag="av",
                                           name=f"avacc_p{p}")
                if t >= 1:
                    assert acc is not None
                    for c in range(QC):
                        nc.tensor.matmul(
                            out=acc[:, 512 * c:512 * (c + 1)],
                            lhsT=vt[:, t - 1, 0:D + 1],
                            rhs=ptk_list[t - 1][:, 512 * c:512 * (c + 1)],
                            start=(t - 1 == 0), stop=False,
                            skip_group_check=True)
            assert acc is not None
            for c in range(QC):
                nc.tensor.matmul(
                    out=acc[:, 512 * c:512 * (c + 1)],
                    lhsT=vt[:, KT - 1, 0:D + 1],
                    rhs=ptk_list[KT - 1][:, 512 * c:512 * (c + 1)],
                    start=False, stop=True, skip_group_check=True)
            prev_acc = acc
        _emit_norm(nc, pools, PAIRS - 1, prev_acc, aps, consts)

    nc.compile()
    return nc


_NC = None


def _get_nc():
    global _NC
    if _NC is None:
        _NC = build()
    return _NC


def kernel(q, k, v, k_length):
    q = np.ascontiguousarray(q, dtype=np.float32)
    k = np.ascontiguousarray(k, dtype=np.float32)
    v = np.ascontiguousarray(v, dtype=np.float32)
    k_length = np.asarray(k_length)

    in_maps = []
    for c in range(N_CORES):
        b = c // 2
        r0 = (c % 2) * ROWS
        in_maps.append({
            "q": np.ascontiguousarray(q[b, r0:r0 + ROWS, :]),
            "k": np.ascontiguousarray(k[b, r0:r0 + ROWS, :]),
            "v": np.ascontiguousarray(v[b, r0:r0 + ROWS, :]),
            "kl": np.array([[np.float32(k_length[b])]], dtype=np.float32),
        })

    nc = _get_nc()
    res = run_bass_kernel_spmd(nc, in_maps, core_ids=list(range(N_CORES)))

    out = np.empty((B, H * D, L), dtype=np.float32)
    for c in range(N_CORES):
        b = c // 2
        r0 = (c % 2) * ROWS
        out[b, r0:r0 + ROWS, :] = res.results[c]["out"]
    return out

